# revision 14
# baseline (speedup 1.0000x reference)
"""Trainium2 Bass kernel for nn_LogisticModel — hybrid DVE + PE variant.

Same math as the pure-DVE variant (kernel_dve.py). The DVE shift pass (scalar_tensor_tensor, stuck
at 1x) is the binding cost of the plain pipeline, while ACT's two
passes (sigmoid, square) are invariant to any decomposition. This
hybrid moves the first 3072 timesteps (37.5% of the data) onto the idle
PE via the transposed-supertile route (see kernel_pe.py): PE computes
    -resid = g + decay*x_prev - x
as accumulating matmuls into PSUM (Wg=I, W1=-I+d*E1, W2[127,0]=d), ACT
squares straight out of PSUM (sign washes in Square), DVE only runs the
final 4x tensor_scalar. The remaining 75% uses the proven DVE pipeline.
PE is ~4x throttled (p-state) but its ~35 us of work hides under the
kernel; its per-supertile square is spread over two pipeline steps so
ACT never waits on throttled matmuls. SQ_FRAC of the normal square
runs on DVE as r*r to balance ACT/DVE at ~55 us each, just under the
DMA floor.

The host splits each core shard at t=3072: x/s/out for t>=3072 stay in
row-major layout (xN carries one extra leading column, so the
boundary never crosses pipelines); t<3072 goes to 3 transposed
supertiles per core (fp16; t=0 handled by simply omitting the W2 term
for the first block). All host work is dtype casts and permutations.
Measured full-input rel err vs the f32 oracle: 1.0e-2 (normal part
dominates; PE part alone is 1.9e-3).
"""

import os
import sys
from contextlib import ExitStack

import numpy as np

for _p in ("/root/.axon_site", "/root/.axon_site/_ro/trn_rl_repo",
           "/root/.axon_site/_ro/pypackages", "/opt/trn_rl_repo"):
    if os.path.isdir(_p) and _p not in sys.path:
        sys.path.append(_p)

import ml_dtypes

import concourse.bass as bass
import concourse.bacc as bacc
import concourse.mybir as mybir
import concourse.tile as tile

BF16 = mybir.dt.bfloat16
F16 = mybir.dt.float16
F32 = mybir.dt.float32
FP8 = mybir.dt.float8e3  # e3m4
P = 128
G = 512          # batch width (matmul free dim)
TSPLIT = 3072    # timesteps handled by the PE route
NSUP = 3         # transposed supertiles per core (TSPLIT/(8*128))
SCOLS = 8 * G    # supertile width

N_CORES = 8
B, T = 4096, 8192

LAST_RESULT = None


def build_module(rows, cols, gain, decay, noise, W=4096, gps_store=True,
                 xn_bufs=4, sn_bufs=3, gn_bufs=3, tn_bufs=4, on_bufs=2):
    """rows=512, cols=8192 logical shard; split at TSPLIT internally."""
    assert rows == 4 * P and cols == T
    ncols = cols - TSPLIT  # normal-route timesteps
    nc = bacc.Bacc()
    xN = nc.declare_dram_parameter("xN", [rows, ncols + 1], BF16,
                                   isOutput=False)
    sN = nc.declare_dram_parameter("sN", [rows, ncols], FP8, isOutput=False)
    outN = nc.declare_dram_parameter("outN", [rows, ncols], BF16,
                                     isOutput=True)
    xT = nc.declare_dram_parameter("xT", [NSUP * P, SCOLS], F16,
                                   isOutput=False)
    sT = nc.declare_dram_parameter("sT", [NSUP * P, SCOLS], FP8,
                                   isOutput=False)
    outT = nc.declare_dram_parameter("outT", [NSUP * P, SCOLS], F16,
                                     isOutput=True)
    w1_in = nc.declare_dram_parameter("w1", [P, P], F16, isOutput=False)
    w2_in = nc.declare_dram_parameter("w2", [P, P], F16, isOutput=False)
    wg_in = nc.declare_dram_parameter("wg", [P, P], F16, isOutput=False)

    log_norm = float(np.log(noise) + 0.5 * np.log(2.0 * np.pi))
    k = float(np.sqrt(0.5) / noise)
    k2 = float(0.5 / noise ** 2)  # for the r*r (DVE-squared) columns
    AF = mybir.ActivationFunctionType
    OP = mybir.AluOpType
    # ACT is the hybrid's ceiling (sigmoid + square + PSUM-square);
    # rebalance by squaring sq_frac of each normal tile on DVE instead.
    SQ_FRAC = 0.65

    def sq_cols(w):
        return (int(w * SQ_FRAC) // P) * P

    # Tile list: normal tiles (r0, t0, w) over t in [TSPLIT, T), tail-
    # tapered; the 2 PE supertiles are placed mid-stream so their
    # throttled matmul chains hide under normal steps instead of
    # stretching the pipeline ramp.
    tiles = []
    n_rb = rows // P
    for rb in range(n_rb):
        widths = ([2048, 1024, 1024, 512, 512] if rb == n_rb - 1
                  else [1024, 1024, 3072] if rb == 0
                  else [1024, 4096])
        t0 = TSPLIT
        for w in widths:
            tiles.append(("n", (rb * P, t0, w)))
            t0 += w
    tiles.insert(2, ("pe", 0))
    tiles.insert(6, ("pe", 1))
    tiles.insert(10, ("pe", 2))
    n = len(tiles)
    st = {}
    pe_x = {}  # supertile u -> its xT tile (outlives st entries)

    with tile.TileContext(nc) as tc, ExitStack() as ctx:
        wp = ctx.enter_context(tc.tile_pool(name="wp", bufs=1))
        xnp = ctx.enter_context(tc.tile_pool(name="xnp", bufs=xn_bufs))
        snp = ctx.enter_context(tc.tile_pool(name="snp", bufs=sn_bufs))
        gnp = ctx.enter_context(tc.tile_pool(name="gnp", bufs=gn_bufs))
        tnp = ctx.enter_context(tc.tile_pool(name="tnp", bufs=tn_bufs))
        onp = ctx.enter_context(tc.tile_pool(name="onp", bufs=on_bufs))
        xtp = ctx.enter_context(tc.tile_pool(name="xtp", bufs=2))
        stp = ctx.enter_context(tc.tile_pool(name="stp", bufs=2))
        gtp = ctx.enter_context(tc.tile_pool(name="gtp", bufs=2))
        qtp = ctx.enter_context(tc.tile_pool(name="qtp", bufs=2))
        otp = ctx.enter_context(tc.tile_pool(name="otp", bufs=2))
        pp = ctx.enter_context(
            tc.tile_pool(name="pp", bufs=4, space=bass.MemorySpace.PSUM))

        # Allocate weight tiles now; issue their DMAs at step 2 so the
        # first data tiles' loads aren't queued behind them (the first
        # matmul consumer sits at step >= 3).
        w1_t = wp.tile([P, P], F16, tag="w1")
        w2_t = wp.tile([P, P], F16, tag="w2")
        wg_t = wp.tile([P, P], F16, tag="wg")

        def load_weights():
            nc.sync.dma_start(w1_t[:], w1_in[:, :])
            nc.sync.dma_start(w2_t[:], w2_in[:, :])
            nc.sync.dma_start(wg_t[:], wg_in[:, :])

        def loads(i):
            kind, info = tiles[i]
            if kind == "pe":
                u = info
                x_t = xtp.tile([P, SCOLS], F16, tag="xt")
                nc.sync.dma_start(x_t[:], xT[u * P:(u + 1) * P, :])
                pe_x[u] = x_t
                s_t = stp.tile([P, SCOLS], FP8, tag="st")
                nc.sync.dma_start(s_t[:], sT[u * P:(u + 1) * P, :])
            else:
                r0, t0, w = info
                c0 = t0 - TSPLIT
                x_t = xnp.tile([P, w + 1], BF16, tag="xn")
                nc.sync.dma_start(x_t[:], xN[r0:r0 + P, c0:c0 + w + 1])
                s_t = snp.tile([P, w], FP8, tag="sn")
                nc.sync.dma_start(s_t[:], sN[r0:r0 + P, c0:c0 + w])
            st[i] = {"x": x_t, "s": s_t}

        def sig(i):
            kind, info = tiles[i]
            if kind == "pe":
                g_t = gtp.tile([P, SCOLS], F16, tag="gt")
            else:
                g_t = gnp.tile([P, info[2]], BF16, tag="gn")
            nc.scalar.activation(g_t[:], st[i]["s"], AF.Sigmoid,
                                 scale=float(gain))
            st[i]["g"] = g_t

        def mms(i):
            u = tiles[i][1]
            x_t, g_t = st[i]["x"], st[i]["g"]
            ps = []
            for q in range(4):  # quarters: 2 blocks each, 2 PSUM banks
                ps_t = pp.tile([P, 2 * G], F32, tag="ps")
                for j in range(2):
                    b = 2 * q + j
                    o_ap = ps_t[:, j * G:(j + 1) * G]
                    have_prev = not (u == 0 and b == 0)
                    nc.tensor.matmul(o_ap, wg_t[:],
                                     g_t[:, b * G:(b + 1) * G],
                                     start=True, stop=False)
                    nc.tensor.matmul(o_ap, w1_t[:],
                                     x_t[:, b * G:(b + 1) * G],
                                     start=False, stop=not have_prev)
                    if have_prev:
                        xprev = (x_t[:, (b - 1) * G:b * G] if b > 0 else
                                 pe_x[u - 1][:, 7 * G:8 * G])
                        nc.tensor.matmul(o_ap, w2_t[:], xprev,
                                         start=False, stop=True)
                ps.append(ps_t)
            st[i]["ps"] = ps

        def stt(i):
            _, t0, w = tiles[i][1]
            x_t = st[i]["x"]
            t_t = tnp.tile([P, w], BF16, tag="tn")
            nc.vector.scalar_tensor_tensor(
                t_t[:], x_t[:, 0:w], -float(decay),
                x_t[:, 1:w + 1], OP.mult, OP.add)
            st[i]["t"] = t_t

        def tt(i):
            t_t = st[i]["t"]
            nc.vector.tensor_tensor(t_t[:], t_t[:], st[i]["g"], OP.subtract)

        def sq(i):
            kind = tiles[i][0]
            if kind == "pe":
                q_t = qtp.tile([P, SCOLS], F16, tag="qt")
                for q in (0, 1):
                    nc.scalar.activation(q_t[:, q * 2 * G:(q + 1) * 2 * G],
                                         st[i]["ps"][q][:], AF.Square,
                                         scale=k)
                st[i]["q"] = q_t
            else:
                w = tiles[i][1][2]
                cs = sq_cols(w)
                t_t = st[i]["t"]
                nc.scalar.activation(t_t[:, cs:w], t_t[:, cs:w],
                                     AF.Square, scale=k)
                if cs:
                    # plain r*r on DVE (2x); the matching ts uses -k2
                    nc.vector.tensor_tensor(t_t[:, 0:cs], t_t[:, 0:cs],
                                            t_t[:, 0:cs], OP.mult)

        def sq_pe_late(i):
            q_t = st[i]["q"]
            for q in (2, 3):
                nc.scalar.activation(q_t[:, q * 2 * G:(q + 1) * 2 * G],
                                     st[i]["ps"][q][:], AF.Square, scale=k)

        def ts_store(i):
            kind, info = tiles[i]
            if kind == "pe":
                u = info
                q_t = st.pop(i)["q"]
                o_t = otp.tile([P, SCOLS], F16, tag="ot")
                nc.vector.tensor_scalar(o_t[:], q_t[:], -1.0, -log_norm,
                                        OP.mult, OP.add)
                dst = outT[u * P:(u + 1) * P, :]
            else:
                r0, t0, w = info
                cs = sq_cols(w)
                t_t = st.pop(i)["t"]
                o_t = onp.tile([P, w], BF16, tag="on")
                if cs:
                    nc.vector.tensor_scalar(o_t[:, 0:cs], t_t[:, 0:cs],
                                            -k2, -log_norm, OP.mult, OP.add)
                nc.vector.tensor_scalar(o_t[:, cs:w], t_t[:, cs:w],
                                        -1.0, -log_norm, OP.mult, OP.add)
                dst = outN[r0:r0 + P, t0 - TSPLIT:t0 - TSPLIT + w]
            if gps_store:
                nc.gpsimd.dma_start(dst, o_t[:])
            else:
                nc.scalar.dma_start(dst, o_t[:])

        for i in range(n + 2):
            if i == 2:
                load_weights()
            if i < n:
                loads(i)
            if 1 <= i < n + 1 and tiles[i - 1][0] == "n":
                tt(i - 1)
            if i < n:
                sig(i)
            if i >= 2 and tiles[i - 2][0] == "pe":
                sq_pe_late(i - 2)
            if i >= 2:
                ts_store(i - 2)
            if i < n:
                if tiles[i][0] == "pe":
                    mms(i)
                else:
                    stt(i)
            if 1 <= i < n + 1:
                sq(i - 1)
    nc.compile()
    return nc


_MODULE_CACHE = {}

BUILD_KW = {}


def _to_dev_pe(a):
    """[512, TSPLIT] -> [NSUP*128, 4096] supertile-major transposed."""
    return np.ascontiguousarray(
        a.T.reshape(NSUP, 8, P, G).transpose(0, 2, 1, 3).reshape(
            NSUP * P, SCOLS))


def _from_dev_pe(a):
    return np.ascontiguousarray(
        a.reshape(NSUP, P, 8, G).transpose(0, 2, 1, 3).reshape(
            TSPLIT, G).T)


def kernel(s, x, gain, decay, noise):
    global LAST_RESULT
    from concourse.bass_utils import run_bass_kernel_spmd

    s = np.asarray(s, dtype=np.float32)
    x = np.asarray(x, dtype=np.float32)
    b, t = s.shape
    rows = b // N_CORES
    assert rows == 512 and t == T

    dw = np.float16(decay)
    w1 = (-np.eye(P) + float(dw) * np.eye(P, k=1)).astype(np.float16)
    w2 = np.zeros((P, P), np.float16)
    w2[P - 1, 0] = dw
    wg = np.eye(P).astype(np.float16)

    key = (rows, t, float(gain), float(decay), float(noise), "hy5") + tuple(
        sorted(BUILD_KW.items()))
    if key not in _MODULE_CACHE:
        _MODULE_CACHE[key] = build_module(
            rows, t, float(gain), float(decay), float(noise), **BUILD_KW)
    nc = _MODULE_CACHE[key]

    in_maps = []
    for i in range(N_CORES):
        sc = s[i * rows:(i + 1) * rows]
        xc = x[i * rows:(i + 1) * rows]
        in_maps.append({
            "xN": np.ascontiguousarray(
                xc[:, TSPLIT - 1:]).astype(ml_dtypes.bfloat16),
            "sN": np.ascontiguousarray(
                sc[:, TSPLIT:]).astype(ml_dtypes.float8_e3m4),
            "xT": _to_dev_pe(xc[:, :TSPLIT].astype(np.float16)),
            "sT": _to_dev_pe(sc[:, :TSPLIT].astype(ml_dtypes.float8_e3m4)),
            "w1": w1, "w2": w2, "wg": wg,
        })
    res = run_bass_kernel_spmd(nc, in_maps, list(range(N_CORES)))
    LAST_RESULT = res

    out = np.empty((b, t), np.float32)
    for i in range(N_CORES):
        r = res.results[i]
        out[i * rows:(i + 1) * rows, :TSPLIT] = _from_dev_pe(
            r["outT"]).astype(np.float32)
        out[i * rows:(i + 1) * rows, TSPLIT:] = r["outN"].astype(np.float32)
    return out
